# revision 29
# baseline (speedup 1.0000x reference)
"""Fused multi-head self-attention (T=2048, B=2, E=1024, H=16) on 8 TRN2 cores.

Sharding: batch*heads across cores — core c handles b = c//4, heads
[(c%4)*4, (c%4)*4+4). Projections are column-split (Wq/Wk/Wv) per core's
heads; Wo is row-split with the cross-core reduction done on the host
during unshard (4 partial [T,E] sums per batch element).

Device kernel (per core, identical SPMD program):
  - qT/kT produced transposed [64*2-pair, T] so scores need no transposes
  - scores computed transposed sT[s,m] = kT.T @ qT, softmax along the
    PSUM free dim is avoided entirely: exp on ScalarE, denominators via a
    ones-column appended to v (row 64 of the AV accumulation), normalize
    by a K=1 broadcast matmul + DVE multiply
  - causal structure exploited by block classification (compile-time):
    fully-masked 128x128 blocks skipped, zero blocks exp'd directly,
    additive blocks get the real mask values added
  - matmuls in bf16 with fp32 PSUM accumulation (keeps the PE clock-gate
    warm; fp32/fp32r matmul modes run ~2-4x slower and stay throttled)
"""
import os
import sys

import numpy as np

for _p in ("/opt/trn_rl_repo", "/root/.axon_site/_ro/trn_rl_repo"):
    if os.path.isdir(_p) and _p not in sys.path:
        sys.path.insert(0, _p)
        break

import concourse.bacc as bacc
import concourse.mybir as mybir
import concourse.tile as tile
from concourse.bass_utils import run_bass_kernel_spmd

f32 = mybir.dt.float32
bf16 = mybir.dt.bfloat16
AF = mybir.ActivationFunctionType

T, B, E, H, HD = 2048, 2, 1024, 16, 64
NCORES = 8
HL = (B * H) // NCORES          # heads per core = 4
J = HL * HD                     # per-core projection width = 256
EC = E // 128                   # e-chunks = 8
SCALE = HD ** -0.5
MCH = 512                       # m-chunk width
NEG_THRESH = -1e8               # "fully masked" threshold

SKIP, ZERO, ADD, ADDBIN = 0, 1, 2, 3

_prog_cache = {}


def _classify_mask(mask):
    """Classify 128x128 blocks of mask[t_query, s_key]."""
    nb = mask.shape[0] // 128
    blocks = mask.reshape(nb, 128, nb, 128)
    all_skip = (blocks <= NEG_THRESH).all(axis=(1, 3))
    all_zero = (blocks == 0.0).all(axis=(1, 3))
    binary = ((blocks == 0.0) | (blocks <= NEG_THRESH)).all(axis=(1, 3))
    cls = np.where(all_skip, SKIP,
                   np.where(all_zero, ZERO, np.where(binary, ADDBIN, ADD)))
    return cls  # [m_block, s_block]


def _build(T_, cls_key):
    cls = np.array(cls_key, dtype=np.int64)
    NB = T_ // 128
    NMC = T_ // MCH
    add_blocks = [(mb, sb) for mb in range(NB) for sb in range(NB)
                  if cls[mb, sb] == ADD]
    add_pos = {blk: i for i, blk in enumerate(add_blocks)}
    n_add = len(add_blocks)
    bin_blocks = [(mb, sb) for mb in range(NB) for sb in range(NB)
                  if cls[mb, sb] == ADDBIN]
    bin_pos = {blk: i for i, blk in enumerate(bin_blocks)}
    n_bin = len(bin_blocks)

    nc = bacc.Bacc("TRN2", target_bir_lowering=False, debug=False)
    xT = nc.declare_dram_parameter("xT", [E, T_], bf16, isOutput=False)
    wqpack = nc.declare_dram_parameter("wqpack", [128, EC * J], bf16,
                                       isOutput=False)
    wkvpack = nc.declare_dram_parameter("wkvpack", [128, 2 * EC * J], bf16,
                                        isOutput=False)
    wopack = nc.declare_dram_parameter("wopack", [128, (J // 128) * E], bf16,
                                       isOutput=False)
    bqp = nc.declare_dram_parameter("bqp", [128, 2], f32, isOutput=False)
    ones1 = nc.declare_dram_parameter("ones1", [1, 64], bf16, isOutput=False)
    msk = nc.declare_dram_parameter("msk", [128, max(n_add, 1) * 128], f32,
                                    isOutput=False)
    tri = nc.declare_dram_parameter("tri", [128, max(n_bin, 1) * 128], bf16,
                                    isOutput=False)
    ident = nc.declare_dram_parameter("ident", [128, 128], bf16, isOutput=False)
    out = nc.declare_dram_parameter("out", [T_, E], f32, isOutput=True)

    with tile.TileContext(nc) as tc:
        with nc.allow_low_precision(reason="bf16 matmuls, fp32 psum"), \
             tc.tile_pool(name="sba", bufs=1) as sba, \
             tc.tile_pool(name="sbw", bufs=1) as sbw, \
             tc.tile_pool(name="ps", bufs=1, space="PSUM") as ps:
            xT_sb = sba.tile([128, EC * T_], bf16)
            wpack_sb = sba.tile([128, 3 * EC * J], bf16)
            wq_sb = wpack_sb[:, 0:EC * J]
            wk_sb = wpack_sb[:, EC * J:2 * EC * J]
            wv_sb = wpack_sb[:, 2 * EC * J:3 * EC * J]
            del_unused = None
            wo_sb = sba.tile([128, (J // 128) * E], bf16)
            qT_sb = sba.tile([128, 2 * T_], bf16)
            kT_sb = sba.tile([128, 2 * T_], bf16)
            v_sb = sba.tile([128, HL * NB * 65], bf16)
            oT_sb = sba.tile([128, 2 * T_], bf16)
            bq_sb = sba.tile([128, 2], f32)
            msk_sb = sba.tile([128, max(n_add, 1) * 128], f32)
            tri_sb = sba.tile([128, max(n_bin, 1) * 128], bf16)
            ones1_sb = sba.tile([1, 64], bf16)
            negc = sba.tile([128, 1], f32)
            nc.vector.memset(negc[:], -100.0)
            ident_sb = sba.tile([128, 128], bf16)
            onef_sb = sba.tile([1, 1], f32)
            nc.vector.memset(onef_sb[:], 1.0)

            # ---- input DMAs (ordered so the first matmul starts ASAP) ----
            nc.sync.dma_start(wq_sb, wqpack[:, :])
            nc.sync.dma_start(xT_sb[:, 0:T_], xT[0:128, :])
            nc.sync.dma_start(wpack_sb[:, EC * J:3 * EC * J], wkvpack[:, :])
            nc.sync.dma_start(bq_sb[:], bqp[:, :])
            nc.sync.dma_start(ones1_sb[:], ones1[:, :])
            for c in range(1, EC):
                nc.sync.dma_start(xT_sb[:, c * T_:(c + 1) * T_],
                                  xT[c * 128:(c + 1) * 128, :])
            nc.sync.dma_start(wo_sb[:], wopack[:, :])
            v_ones_view = v_sb[:].rearrange("p (x c) -> p x c", c=65)[:, :, 64:65]
            nc.vector.memset(v_ones_view, 1.0)
            if n_add:
                nc.sync.dma_start(msk_sb[:], msk[:, :])
            if n_bin:
                nc.sync.dma_start(tri_sb[:], tri[:, :])
            nc.sync.dma_start(ident_sb[:], ident[:, :])

            # ---- projection groups (n=0 / v 0..3 upfront; rest are
            # attention-phase PE filler) ----
            def qk_group(nn, u, wsb, dst, biased):
                psq = ps.tile([128, 512], f32, tag="big", bufs=2)
                for c in range(EC):
                    nc.tensor.matmul(
                        psq[:],
                        wsb[:, c * J + u * 128: c * J + (u + 1) * 128],
                        xT_sb[:, c * T_ + nn * 512: c * T_ + nn * 512 + 512],
                        start=(c == 0), stop=(c == EC - 1))
                dslc = dst[:, u * T_ + nn * 512: u * T_ + nn * 512 + 512]
                if biased:
                    nc.vector.tensor_scalar_add(dslc, psq[:], bq_sb[:, u:u + 1])
                else:
                    nc.vector.tensor_copy(dslc, psq[:])

            def v_group(i):
                psv = ps.tile([128, 512], f32, tag="big", bufs=2)
                for c in range(EC):
                    nc.tensor.matmul(
                        psv[:, 0:J],
                        xT_sb[:, c * T_ + i * 128: c * T_ + i * 128 + 128],
                        wv_sb[:, c * J:(c + 1) * J],
                        start=(c == 0), stop=(c == EC - 1))
                for h in range(HL):
                    nc.vector.tensor_copy(
                        v_sb[:, (h * NB + i) * 65:(h * NB + i) * 65 + 64],
                        psv[:, h * 64:(h + 1) * 64])

            from collections import deque
            v_upfront = min(4, NB)
            for u in range(2):
                for wsb, dst, biased in ((wq_sb, qT_sb, True),
                                         (wk_sb, kT_sb, False)):
                    qk_group(0, u, wsb, dst, biased)
            for i in range(v_upfront):
                v_group(i)

            def _qk_thunk(nn, u, wsb, dst, biased):
                return lambda: qk_group(nn, u, wsb, dst, biased)

            def _v_thunk(i):
                return lambda: v_group(i)

            fill = deque()
            for nn in range(1, NMC):
                for u in range(2):
                    for wsb, dst, biased in ((wq_sb, qT_sb, True),
                                             (wk_sb, kT_sb, False)):
                        fill.append((nn, _qk_thunk(nn, u, wsb, dst, biased)))
                for i in range(4 * nn, min(4 * nn + 4, NB)):
                    fill.append((nn, _v_thunk(i)))
            for i in range(4 * NMC, NB):
                fill.append((NMC - 1, _v_thunk(i)))

            # ---- attention: head-pair concurrent scores (row-groups 0-1 vs
            # 2-3), split-K AV halves, pipelined normalize, deferred out-proj
            def s_loop_pair(n, u, side_work=(), fill_q=None):
                side_work = list(side_work)
                hA, hB = 2 * u, 2 * u + 1
                stiles = [i for i in range(NB)
                          if any(cls[n * 4 + k, i] != SKIP for k in range(4))]
                psoA = ps.tile([128, 512], f32, tag="attno", bufs=4)
                psoB = ps.tile([128, 512], f32, tag="attno", bufs=4)
                qA = qT_sb[0:64, u * T_ + n * 512: u * T_ + n * 512 + 512]
                qB = qT_sb[64:128, u * T_ + n * 512: u * T_ + n * 512 + 512]
                last = len(stiles) - 1
                for idx, i in enumerate(stiles):
                    pss = ps.tile([128, 1024], f32, tag="big", bufs=2)
                    kA = kT_sb[0:64, u * T_ + i * 128: u * T_ + i * 128 + 128]
                    kB = kT_sb[64:128, u * T_ + i * 128: u * T_ + i * 128 + 128]
                    nc.tensor.matmul(pss[:, 0:512], kA, qA,
                                     start=True, stop=True, skip_group_check=True)
                    nc.tensor.matmul(pss[:, 512:1024], kB, qB,
                                     start=True, stop=True, skip_group_check=True)
                    for k in range(4):
                        if cls[n * 4 + k, i] == ADD:
                            pos = add_pos[(n * 4 + k, i)]
                            mblk = msk_sb[:, pos * 128:(pos + 1) * 128]
                            for off in (0, 512):
                                nc.vector.tensor_add(
                                    pss[:, off + k * 128: off + (k + 1) * 128],
                                    pss[:, off + k * 128: off + (k + 1) * 128],
                                    mblk)
                    pt = sbw.tile([128, 1024], bf16, tag="pt", bufs=4)
                    # exp over runs of equal skip-ness (same for both heads)
                    runs = []
                    k = 0
                    while k < 4:
                        k1 = k
                        skipk = cls[n * 4 + k, i] == SKIP
                        while k1 < 4 and (cls[n * 4 + k1, i] == SKIP) == skipk:
                            k1 += 1
                        runs.append((k, k1, skipk))
                        k = k1
                    if runs == [(0, 4, False)]:
                        nc.scalar.activation(pt[:], pss[:], AF.Exp)
                    else:
                        for k, k1, skipk in runs:
                            for off in (0, 512):
                                src = pss[:, off + k * 128: off + k1 * 128]
                                dst = pt[:, off + k * 128: off + k1 * 128]
                                if skipk:
                                    nc.gpsimd.memset(dst, 0.0)
                                else:
                                    nc.scalar.activation(dst, src, AF.Exp)
                    for k in range(4):
                        if cls[n * 4 + k, i] == ADDBIN:
                            pos = bin_pos[(n * 4 + k, i)]
                            tblk = tri_sb[:, pos * 128:(pos + 1) * 128]
                            for off in (0, 512):
                                nc.gpsimd.tensor_mul(
                                    pt[:, off + k * 128: off + (k + 1) * 128],
                                    pt[:, off + k * 128: off + (k + 1) * 128],
                                    tblk)
                    if side_work:
                        side_work.pop(0)()
                    elif fill_q:
                        fill_q.popleft()[1]()
                    for pso_, h, off in ((psoA, hA, 0), (psoB, hB, 512)):
                        strip = v_sb[:, (h * NB + i) * 65:(h * NB + i) * 65 + 65]
                        nc.tensor.matmul(
                            pso_[0:65, :], strip[:, :],
                            pt[:, off:off + 512],
                            start=(idx == 0), stop=(idx == last),
                            skip_group_check=True)
                while side_work:
                    side_work.pop(0)()
                return psoA, psoB

            def normalize_early_pair(psoA, psoB):
                """Compute 1/rowsums for both heads of a pair with the sums
                transposed onto partitions via tiny matmuls, so the DVE
                reciprocal is 8 elems/lane instead of 512 on one lane.
                Returns (recip_sb [1,1024] bf16, work-thunks)."""
                sums_sb = sbw.tile([1, 1024], f32, tag="sums", bufs=3)
                rT = sbw.tile([128, 8], bf16, tag="rT", bufs=3)
                recip = sbw.tile([1, 1024], bf16, tag="recip", bufs=3)

                def t1():
                    nc.scalar.copy(sums_sb[0:1, 0:512], psoA[64:65, :])
                    nc.scalar.copy(sums_sb[0:1, 512:1024], psoB[64:65, :])
                    pst = ps.tile([128, 8], f32, tag="big", bufs=2)
                    for c in range(8):
                        nc.tensor.matmul(
                            pst[:, c:c + 1],
                            sums_sb[0:1, c * 128:(c + 1) * 128],
                            onef_sb[:, :], start=True, stop=True,
                            skip_group_check=True)
                    nc.vector.reciprocal(rT[:], pst[:])
                    return pst

                def t2(pst):
                    psr = ps.tile([1, 1024], f32, tag="big", bufs=2)
                    for c in range(8):
                        nc.tensor.matmul(
                            psr[0:1, c * 128:(c + 1) * 128],
                            rT[:, c:c + 1], ident_sb[:, :],
                            start=True, stop=True, skip_group_check=True)
                    nc.vector.tensor_copy(recip[:], psr[:])
                box = {}

                def w1():
                    box['pst'] = t1()

                def w2():
                    t2(box['pst'])
                return recip, [w1, w2]

            def normalize_late(n, h, pso_, recip):
                u, poff = h >> 1, (h & 1) * 64
                psb = ps.tile([128, 512], f32, tag="big", bufs=2)
                nc.tensor.matmul(psb[0:64, :], ones1_sb[:], recip[:],
                                 start=True, stop=True, skip_group_check=True)
                rb = sbw.tile([64, 512], f32, tag="rb", bufs=2)
                nc.scalar.copy(rb[:], psb[0:64, :])
                nc.vector.tensor_mul(
                    oT_sb[poff:poff + 64, u * T_ + n * 512: u * T_ + n * 512 + 512],
                    pso_[0:64, :], rb[:])

            def normalize_late_thunk(n, h, pso_, recip):
                return lambda: normalize_late(n, h, pso_, recip)

            def out_proj_group(m16, eh):
                pso = ps.tile([128, 512], f32, tag="big", bufs=2)
                for jc in range(J // 128):
                    nc.tensor.matmul(
                        pso[:],
                        oT_sb[:, jc * T_ + m16 * 128: jc * T_ + m16 * 128 + 128],
                        wo_sb[:, jc * E + eh * 512: jc * E + eh * 512 + 512],
                        start=(jc == 0), stop=(jc == J // 128 - 1),
                        skip_group_check=True)
                ob = sbw.tile([128, 512], f32, tag="ob", bufs=3)
                nc.scalar.copy(ob[:], pso[:])
                nc.sync.dma_start(
                    out[m16 * 128:(m16 + 1) * 128,
                        eh * 512:(eh + 1) * 512], ob[:])

            def out_proj_thunks(n):
                def grp(m16, eh):
                    return lambda: out_proj_group(m16, eh)
                return [grp(m16, eh) for m16 in range(n * 4, n * 4 + 4)
                        for eh in range(E // 512)]

            def out_proj(n):
                for w in out_proj_thunks(n):
                    w()

            prevpair = None
            carry = []
            for n in range(NMC):
                for u in range(2):
                    work = []
                    if prevpair is not None:
                        pn, pu, pA, pB = prevpair
                        rAB, wAB = normalize_early_pair(pA, pB)
                        work = list(wAB)
                        work.append(normalize_late_thunk(pn, 2 * pu, pA,
                                                         rAB[0:1, 0:512]))
                        work.append(normalize_late_thunk(pn, 2 * pu + 1, pB,
                                                         rAB[0:1, 512:1024]))
                    work += carry
                    carry = []
                    while fill and fill[0][0] <= n:
                        fill.popleft()[1]()
                    psoA, psoB = s_loop_pair(n, u, work, fill)
                    if prevpair is not None and pu == 1:
                        carry = out_proj_thunks(pn)
                    prevpair = (n, u, psoA, psoB)
            for w in carry:
                w()
            pn, pu, pA, pB = prevpair
            rAB, wAB = normalize_early_pair(pA, pB)
            for w in wAB:
                w()
            normalize_late(pn, 2 * pu, pA, rAB[0:1, 0:512])
            normalize_late(pn, 2 * pu + 1, pB, rAB[0:1, 512:1024])
            out_proj(NMC - 1)

    nc.compile()
    return nc


def _get_program(T_, cls):
    key = (T_, tuple(map(tuple, cls.tolist())))
    if key not in _prog_cache:
        _prog_cache[key] = _build(T_, key[1])
    return _prog_cache[key]


def _numpy_ref(query, attn_mask, key_padding_mask, Wq, bq, Wk, bk, Wv, bv,
               Wo, bo):
    """Exact-semantics fallback (mirrors reference.py in numpy)."""
    q = (query @ Wq.T + bq) * SCALE
    k = query @ Wk.T + bk
    v = query @ Wv.T + bv

    def shp(x):
        return x.reshape(T, B * H, HD).transpose(1, 0, 2)

    q, k, v = shp(q), shp(k), shp(v)
    w = np.einsum('bth,bsh->bts', q, k).reshape(B, H, T, T) + attn_mask
    w = np.where(key_padding_mask[:, None, None, :], -np.inf, w)
    w = w - w.max(axis=-1, keepdims=True)
    ew = np.exp(w)
    p = (ew / ew.sum(axis=-1, keepdims=True)).reshape(B * H, T, T)
    o = np.einsum('bts,bsh->bth', p, v.reshape(B * H, T, HD))
    o = o.transpose(1, 0, 2).reshape(T, B, E)
    return (o @ Wo.T + bo).astype(np.float32)


def _prep_inputs(query, attn_mask, Wq, bq, Wk, Wv, Wo, cls):
    """Build the 8 per-core input maps."""
    import ml_dtypes
    bf = ml_dtypes.bfloat16
    add_blocks = [(mb, sb) for mb in range(T // 128) for sb in range(T // 128)
                  if cls[mb, sb] == ADD]
    n_add = len(add_blocks)
    if n_add:
        mskp = np.empty((128, n_add * 128), np.float32)
        for i, (mb, sb) in enumerate(add_blocks):
            blk = attn_mask[mb * 128:(mb + 1) * 128, sb * 128:(sb + 1) * 128]
            mskp[:, i * 128:(i + 1) * 128] = np.ascontiguousarray(blk.T)
    else:
        mskp = np.zeros((128, 128), np.float32)
    bin_blocks = [(mb, sb) for mb in range(T // 128) for sb in range(T // 128)
                  if cls[mb, sb] == ADDBIN]
    if bin_blocks:
        trip = np.empty((128, len(bin_blocks) * 128), bf)
        for i, (mb, sb) in enumerate(bin_blocks):
            blk = attn_mask[mb * 128:(mb + 1) * 128, sb * 128:(sb + 1) * 128]
            trip[:, i * 128:(i + 1) * 128] = (blk.T == 0.0).astype(bf)
    else:
        trip = np.zeros((128, 128), bf)
    ones1 = np.ones((1, 64), bf)
    identity = np.eye(128, dtype=np.float32).astype(bf)

    in_maps = []
    for core in range(NCORES):
        b = core // (NCORES // B)
        jsl = slice((core % (NCORES // B)) * J, (core % (NCORES // B)) * J + J)
        EC_, J_ = E // 128, J

        def sb_layout(wT):  # [E, J] -> SBUF [128, EC*J]
            return np.ascontiguousarray(
                wT.reshape(EC_, 128, J_).transpose(1, 0, 2).reshape(128, EC_ * J_))

        xT_c = np.ascontiguousarray(query[:, b, :].T).astype(bf)
        wq_l = sb_layout((Wq[jsl, :] * np.float32(SCALE)).T)
        wk_l = sb_layout(Wk[jsl, :].T)
        wv_l = sb_layout(Wv[jsl, :].T)
        wqpack = np.ascontiguousarray(wq_l).astype(bf)
        wkvpack = np.concatenate([wk_l, wv_l], axis=1).astype(bf)
        woT = Wo[:, jsl].T  # [J, E]
        wopack = np.ascontiguousarray(
            woT.reshape(J_ // 128, 128, E).transpose(1, 0, 2)
            .reshape(128, (J_ // 128) * E)).astype(bf)
        bq_c = np.ascontiguousarray(
            (bq[jsl] * np.float32(SCALE)).reshape(2, 128).T)
        in_maps.append({
            "xT": xT_c, "wqpack": wqpack, "wkvpack": wkvpack,
            "wopack": wopack, "bqp": bq_c, "ones1": ones1, "msk": mskp,
            "tri": trip, "ident": identity,
        })
    return in_maps


def _kernel_impl(inputs, trace=False, **run_kwargs):
    query = np.asarray(inputs["query"], np.float32)
    attn_mask = np.asarray(inputs["attn_mask"], np.float32)
    kpm = np.asarray(inputs["key_padding_mask"])
    Wq = np.asarray(inputs["Wq"], np.float32)
    bq = np.asarray(inputs["bq"], np.float32)
    Wk = np.asarray(inputs["Wk"], np.float32)
    bk = np.asarray(inputs["bk"], np.float32)
    Wv = np.asarray(inputs["Wv"], np.float32)
    bv = np.asarray(inputs["bv"], np.float32)
    Wo = np.asarray(inputs["Wo"], np.float32)
    bo = np.asarray(inputs["bo"], np.float32)

    # Fast path requires: no key padding, no fully-masked rows, block-
    # classifiable mask with a modest number of additive blocks, and no
    # bk dependence issue (bk shifts are softmax-invariant, always ok).
    cls = _classify_mask(attn_mask)
    fallback = (
        kpm.any()
        or (attn_mask.max(axis=1) <= NEG_THRESH).any()
        or (cls == ADD).sum() > 24 or (cls == ADDBIN).sum() > 24
        or np.isnan(attn_mask).any()
    )
    if fallback:
        return _numpy_ref(query, attn_mask, kpm, Wq, bq, Wk, bk, Wv, bv,
                          Wo, bo), None

    nc = _get_program(T, cls)
    in_maps = _prep_inputs(query, attn_mask, Wq, bq, Wk, Wv, Wo, cls)
    res = run_bass_kernel_spmd(nc, in_maps, core_ids=list(range(NCORES)),
                               trace=trace, **run_kwargs)

    # unshard: sum the 4 row-split partials per batch element (the Wo
    # all-reduce), then add bo and the bv contribution (sum_s p = 1).
    bo_total = bo + Wo @ bv
    out = np.empty((T, B, E), np.float32)
    gsz = NCORES // B
    for b in range(B):
        acc = res.results[b * gsz]["out"].astype(np.float32)
        for c in range(b * gsz + 1, (b + 1) * gsz):
            acc = acc + res.results[c]["out"]
        out[:, b, :] = acc + bo_total[None, :]
    return out, res


def kernel(**inputs):
    out, _ = _kernel_impl(inputs, trace=False)
    return out


# revision 30
# speedup vs baseline: 1.1327x; 1.1327x over previous
"""Fused multi-head self-attention (T=2048, B=2, E=1024, H=16) on 8 TRN2 cores.

Sharding: batch*heads across cores — core c handles b = c//4, heads
[(c%4)*4, (c%4)*4+4). Projections are column-split (Wq/Wk/Wv) per core's
heads; Wo is row-split with the cross-core reduction done on the host
during unshard (4 partial [T,E] sums per batch element).

Device kernel (per core, identical SPMD program):
  - qT/kT produced transposed [64*2-pair, T] so scores need no transposes
  - scores computed transposed sT[s,m] = kT.T @ qT, softmax along the
    PSUM free dim is avoided entirely: exp on ScalarE, denominators via a
    ones-column appended to v (row 64 of the AV accumulation), normalize
    by a K=1 broadcast matmul + DVE multiply
  - causal structure exploited by block classification (compile-time):
    fully-masked 128x128 blocks skipped, zero blocks exp'd directly,
    additive blocks get the real mask values added
  - matmuls in bf16 with fp32 PSUM accumulation (keeps the PE clock-gate
    warm; fp32/fp32r matmul modes run ~2-4x slower and stay throttled)
"""
import os
import sys

import numpy as np

for _p in ("/opt/trn_rl_repo", "/root/.axon_site/_ro/trn_rl_repo"):
    if os.path.isdir(_p) and _p not in sys.path:
        sys.path.insert(0, _p)
        break

import concourse.bacc as bacc
import concourse.mybir as mybir
import concourse.tile as tile
from concourse.bass_utils import run_bass_kernel_spmd

f32 = mybir.dt.float32
bf16 = mybir.dt.bfloat16
AF = mybir.ActivationFunctionType

T, B, E, H, HD = 2048, 2, 1024, 16, 64
NCORES = 8
HL = (B * H) // NCORES          # heads per core = 4
J = HL * HD                     # per-core projection width = 256
EC = E // 128                   # e-chunks = 8
SCALE = HD ** -0.5
MCH = 512                       # m-chunk width
NEG_THRESH = -1e8               # "fully masked" threshold

SKIP, ZERO, ADD, ADDBIN = 0, 1, 2, 3

_prog_cache = {}


def _classify_mask(mask):
    """Classify 128x128 blocks of mask[t_query, s_key]."""
    nb = mask.shape[0] // 128
    blocks = mask.reshape(nb, 128, nb, 128)
    all_skip = (blocks <= NEG_THRESH).all(axis=(1, 3))
    all_zero = (blocks == 0.0).all(axis=(1, 3))
    binary = ((blocks == 0.0) | (blocks <= NEG_THRESH)).all(axis=(1, 3))
    cls = np.where(all_skip, SKIP,
                   np.where(all_zero, ZERO, np.where(binary, ADDBIN, ADD)))
    return cls  # [m_block, s_block]


def _build(T_, cls_key):
    cls = np.array(cls_key, dtype=np.int64)
    NB = T_ // 128
    NMC = T_ // MCH
    add_blocks = [(mb, sb) for mb in range(NB) for sb in range(NB)
                  if cls[mb, sb] == ADD]
    add_pos = {blk: i for i, blk in enumerate(add_blocks)}
    n_add = len(add_blocks)
    bin_blocks = [(mb, sb) for mb in range(NB) for sb in range(NB)
                  if cls[mb, sb] == ADDBIN]
    bin_pos = {blk: i for i, blk in enumerate(bin_blocks)}
    n_bin = len(bin_blocks)

    nc = bacc.Bacc("TRN2", target_bir_lowering=False, debug=False)
    xT = nc.declare_dram_parameter("xT", [E, T_], bf16, isOutput=False)
    wqpack = nc.declare_dram_parameter("wqpack", [128, EC * J], bf16,
                                       isOutput=False)
    wkvpack = nc.declare_dram_parameter("wkvpack", [128, 2 * EC * J], bf16,
                                        isOutput=False)
    wopack = nc.declare_dram_parameter("wopack", [128, (J // 128) * E], bf16,
                                       isOutput=False)
    bqp = nc.declare_dram_parameter("bqp", [128, 2], f32, isOutput=False)
    ones1 = nc.declare_dram_parameter("ones1", [1, 64], bf16, isOutput=False)
    msk = nc.declare_dram_parameter("msk", [128, max(n_add, 1) * 128], f32,
                                    isOutput=False)
    tri = nc.declare_dram_parameter("tri", [128, max(n_bin, 1) * 128], bf16,
                                    isOutput=False)
    ident = nc.declare_dram_parameter("ident", [128, 128], bf16, isOutput=False)
    out = nc.declare_dram_parameter("out", [T_, E], f32, isOutput=True)

    with tile.TileContext(nc) as tc:
        with nc.allow_low_precision(reason="bf16 matmuls, fp32 psum"), \
             tc.tile_pool(name="sba", bufs=1) as sba, \
             tc.tile_pool(name="sbw", bufs=1) as sbw, \
             tc.tile_pool(name="ps", bufs=1, space="PSUM") as ps:
            xT_sb = sba.tile([128, EC * T_], bf16)
            wpack_sb = sba.tile([128, 3 * EC * J], bf16)
            wq_sb = wpack_sb[:, 0:EC * J]
            wk_sb = wpack_sb[:, EC * J:2 * EC * J]
            wv_sb = wpack_sb[:, 2 * EC * J:3 * EC * J]
            del_unused = None
            wo_sb = sba.tile([128, (J // 128) * E], bf16)
            qT_sb = sba.tile([128, 2 * T_], bf16)
            kT_sb = sba.tile([128, 2 * T_], bf16)
            v_sb = sba.tile([128, HL * NB * 65], bf16)
            oT_sb = sba.tile([128, 2 * T_], bf16)
            bq_sb = sba.tile([128, 2], f32)
            msk_sb = sba.tile([128, max(n_add, 1) * 128], f32)
            tri_sb = sba.tile([128, max(n_bin, 1) * 128], bf16)
            ones1_sb = sba.tile([1, 64], bf16)
            negc = sba.tile([128, 1], f32)
            nc.vector.memset(negc[:], -100.0)
            ident_sb = sba.tile([128, 128], bf16)
            onef_sb = sba.tile([1, 1], f32)
            nc.vector.memset(onef_sb[:], 1.0)

            # ---- input DMAs (ordered so the first matmul starts ASAP) ----
            nc.sync.dma_start(wq_sb, wqpack[:, :])
            nc.sync.dma_start(xT_sb[:, 0:T_], xT[0:128, :])
            nc.sync.dma_start(wpack_sb[:, EC * J:3 * EC * J], wkvpack[:, :])
            nc.sync.dma_start(bq_sb[:], bqp[:, :])
            nc.sync.dma_start(ones1_sb[:], ones1[:, :])
            for c in range(1, EC):
                nc.sync.dma_start(xT_sb[:, c * T_:(c + 1) * T_],
                                  xT[c * 128:(c + 1) * 128, :])
            nc.sync.dma_start(wo_sb[:], wopack[:, :])
            v_ones_view = v_sb[:].rearrange("p (x c) -> p x c", c=65)[:, :, 64:65]
            nc.vector.memset(v_ones_view, 1.0)
            if n_add:
                nc.sync.dma_start(msk_sb[:], msk[:, :])
            if n_bin:
                nc.sync.dma_start(tri_sb[:], tri[:, :])
            nc.sync.dma_start(ident_sb[:], ident[:, :])

            # ---- projection groups (n=0 / v 0..3 upfront; rest are
            # attention-phase PE filler) ----
            def qk_group(nn, u, wsb, dst, biased):
                psq = ps.tile([128, 512], f32, tag="big", bufs=2)
                for c in range(EC):
                    nc.tensor.matmul(
                        psq[:],
                        wsb[:, c * J + u * 128: c * J + (u + 1) * 128],
                        xT_sb[:, c * T_ + nn * 512: c * T_ + nn * 512 + 512],
                        start=(c == 0), stop=(c == EC - 1))
                dslc = dst[:, u * T_ + nn * 512: u * T_ + nn * 512 + 512]
                if biased:
                    nc.vector.tensor_scalar_add(dslc, psq[:], bq_sb[:, u:u + 1])
                else:
                    nc.vector.tensor_copy(dslc, psq[:])

            def v_group(i):
                psv = ps.tile([128, 512], f32, tag="big", bufs=2)
                for c in range(EC):
                    nc.tensor.matmul(
                        psv[:, 0:J],
                        xT_sb[:, c * T_ + i * 128: c * T_ + i * 128 + 128],
                        wv_sb[:, c * J:(c + 1) * J],
                        start=(c == 0), stop=(c == EC - 1))
                for h in range(HL):
                    nc.vector.tensor_copy(
                        v_sb[:, (h * NB + i) * 65:(h * NB + i) * 65 + 64],
                        psv[:, h * 64:(h + 1) * 64])

            from collections import deque
            v_upfront = min(4, NB)
            for u in range(2):
                for wsb, dst, biased in ((wq_sb, qT_sb, True),
                                         (wk_sb, kT_sb, False)):
                    qk_group(0, u, wsb, dst, biased)
            for i in range(v_upfront):
                v_group(i)

            def _qk_thunk(nn, u, wsb, dst, biased):
                return lambda: qk_group(nn, u, wsb, dst, biased)

            def _v_thunk(i):
                return lambda: v_group(i)

            fill = deque()
            for nn in range(1, NMC):
                for u in range(2):
                    for wsb, dst, biased in ((wq_sb, qT_sb, True),
                                             (wk_sb, kT_sb, False)):
                        fill.append((nn, _qk_thunk(nn, u, wsb, dst, biased)))
                for i in range(4 * nn, min(4 * nn + 4, NB)):
                    fill.append((nn, _v_thunk(i)))
            for i in range(4 * NMC, NB):
                fill.append((NMC - 1, _v_thunk(i)))

            # ---- attention: head-pair concurrent scores (row-groups 0-1 vs
            # 2-3), split-K AV halves, pipelined normalize, deferred out-proj
            def s_loop_pair(n, u, side_work=(), fill_q=None):
                side_work = list(side_work)
                hA, hB = 2 * u, 2 * u + 1
                stiles = [i for i in range(NB)
                          if any(cls[n * 4 + k, i] != SKIP for k in range(4))]
                psoA = ps.tile([128, 512], f32, tag="attno", bufs=4)
                psoB = ps.tile([128, 512], f32, tag="attno", bufs=4)
                qA = qT_sb[0:64, u * T_ + n * 512: u * T_ + n * 512 + 512]
                qB = qT_sb[64:128, u * T_ + n * 512: u * T_ + n * 512 + 512]
                last = len(stiles) - 1
                for idx, i in enumerate(stiles):
                    pss = ps.tile([128, 1024], f32, tag="big", bufs=2)
                    kA = kT_sb[0:64, u * T_ + i * 128: u * T_ + i * 128 + 128]
                    kB = kT_sb[64:128, u * T_ + i * 128: u * T_ + i * 128 + 128]
                    nc.tensor.matmul(pss[:, 0:512], kA, qA,
                                     start=True, stop=True, skip_group_check=True)
                    nc.tensor.matmul(pss[:, 512:1024], kB, qB,
                                     start=True, stop=True, skip_group_check=True)
                    for k in range(4):
                        if cls[n * 4 + k, i] == ADD:
                            pos = add_pos[(n * 4 + k, i)]
                            mblk = msk_sb[:, pos * 128:(pos + 1) * 128]
                            for off in (0, 512):
                                nc.vector.tensor_add(
                                    pss[:, off + k * 128: off + (k + 1) * 128],
                                    pss[:, off + k * 128: off + (k + 1) * 128],
                                    mblk)
                    pt = sbw.tile([128, 1024], bf16, tag="pt", bufs=4)
                    # exp over runs of equal skip-ness (same for both heads)
                    runs = []
                    k = 0
                    while k < 4:
                        k1 = k
                        skipk = cls[n * 4 + k, i] == SKIP
                        while k1 < 4 and (cls[n * 4 + k1, i] == SKIP) == skipk:
                            k1 += 1
                        runs.append((k, k1, skipk))
                        k = k1
                    if runs == [(0, 4, False)]:
                        nc.scalar.activation(pt[:], pss[:], AF.Exp)
                    else:
                        for k, k1, skipk in runs:
                            for off in (0, 512):
                                src = pss[:, off + k * 128: off + k1 * 128]
                                dst = pt[:, off + k * 128: off + k1 * 128]
                                if skipk:
                                    nc.gpsimd.memset(dst, 0.0)
                                else:
                                    nc.scalar.activation(dst, src, AF.Exp)
                    for k in range(4):
                        if cls[n * 4 + k, i] == ADDBIN:
                            pos = bin_pos[(n * 4 + k, i)]
                            tblk = tri_sb[:, pos * 128:(pos + 1) * 128]
                            for off in (0, 512):
                                nc.gpsimd.tensor_mul(
                                    pt[:, off + k * 128: off + (k + 1) * 128],
                                    pt[:, off + k * 128: off + (k + 1) * 128],
                                    tblk)
                    if side_work:
                        side_work.pop(0)()
                    elif fill_q:
                        fill_q.popleft()[1]()
                    for pso_, h, off in ((psoA, hA, 0), (psoB, hB, 512)):
                        strip = v_sb[:, (h * NB + i) * 65:(h * NB + i) * 65 + 65]
                        nc.tensor.matmul(
                            pso_[0:65, :], strip[:, :],
                            pt[:, off:off + 512],
                            start=(idx == 0), stop=(idx == last),
                            skip_group_check=True)
                while side_work:
                    side_work.pop(0)()
                return psoA, psoB

            def normalize_early_pair(psoA, psoB):
                """Return (recip [1,1024] bf16, work-thunks): 128-wide DVE
                reciprocal chunks interleaved into the next s-loop so they
                never head-of-line-block the DVE queue."""
                recf = sbw.tile([1, 1024], f32, tag="recf", bufs=3)
                recip = sbw.tile([1, 1024], bf16, tag="recip", bufs=3)

                def chunk(j):
                    pso_ = psoA if j < 4 else psoB
                    jj = j % 4

                    def go():
                        nc.vector.reciprocal(
                            recf[0:1, j * 128:(j + 1) * 128],
                            pso_[64:65, jj * 128:(jj + 1) * 128])
                        if j == 7:
                            nc.vector.tensor_copy(recip[:], recf[:])
                    return go
                return recip, [chunk(j) for j in range(8)]

            def normalize_late(n, h, pso_, recip):
                u, poff = h >> 1, (h & 1) * 64
                psb = ps.tile([128, 512], f32, tag="big", bufs=2)
                nc.tensor.matmul(psb[0:64, :], ones1_sb[:], recip[:],
                                 start=True, stop=True, skip_group_check=True)
                rb = sbw.tile([64, 512], f32, tag="rb", bufs=2)
                nc.scalar.copy(rb[:], psb[0:64, :])
                nc.vector.tensor_mul(
                    oT_sb[poff:poff + 64, u * T_ + n * 512: u * T_ + n * 512 + 512],
                    pso_[0:64, :], rb[:])

            def normalize_late_thunk(n, h, pso_, recip):
                return lambda: normalize_late(n, h, pso_, recip)

            def out_proj_group(m16, eh):
                pso = ps.tile([128, 512], f32, tag="big", bufs=2)
                for jc in range(J // 128):
                    nc.tensor.matmul(
                        pso[:],
                        oT_sb[:, jc * T_ + m16 * 128: jc * T_ + m16 * 128 + 128],
                        wo_sb[:, jc * E + eh * 512: jc * E + eh * 512 + 512],
                        start=(jc == 0), stop=(jc == J // 128 - 1),
                        skip_group_check=True)
                ob = sbw.tile([128, 512], f32, tag="ob", bufs=3)
                nc.scalar.copy(ob[:], pso[:])
                nc.sync.dma_start(
                    out[m16 * 128:(m16 + 1) * 128,
                        eh * 512:(eh + 1) * 512], ob[:])

            def out_proj_thunks(n):
                def grp(m16, eh):
                    return lambda: out_proj_group(m16, eh)
                return [grp(m16, eh) for m16 in range(n * 4, n * 4 + 4)
                        for eh in range(E // 512)]

            def out_proj(n):
                for w in out_proj_thunks(n):
                    w()

            prevpair = None
            carry = []
            for n in range(NMC):
                for u in range(2):
                    work = []
                    if prevpair is not None:
                        pn, pu, pA, pB = prevpair
                        rAB, wAB = normalize_early_pair(pA, pB)
                        work = list(wAB)
                        work.append(normalize_late_thunk(pn, 2 * pu, pA,
                                                         rAB[0:1, 0:512]))
                        work.append(normalize_late_thunk(pn, 2 * pu + 1, pB,
                                                         rAB[0:1, 512:1024]))
                    work += carry
                    carry = []
                    while fill and fill[0][0] <= n:
                        fill.popleft()[1]()
                    psoA, psoB = s_loop_pair(n, u, work, fill)
                    if prevpair is not None and pu == 1:
                        carry = out_proj_thunks(pn)
                    prevpair = (n, u, psoA, psoB)
            for w in carry:
                w()
            pn, pu, pA, pB = prevpair
            rAB, wAB = normalize_early_pair(pA, pB)
            for w in wAB:
                w()
            normalize_late(pn, 2 * pu, pA, rAB[0:1, 0:512])
            normalize_late(pn, 2 * pu + 1, pB, rAB[0:1, 512:1024])
            out_proj(NMC - 1)

    nc.compile()
    return nc


def _get_program(T_, cls):
    key = (T_, tuple(map(tuple, cls.tolist())))
    if key not in _prog_cache:
        _prog_cache[key] = _build(T_, key[1])
    return _prog_cache[key]


def _numpy_ref(query, attn_mask, key_padding_mask, Wq, bq, Wk, bk, Wv, bv,
               Wo, bo):
    """Exact-semantics fallback (mirrors reference.py in numpy)."""
    q = (query @ Wq.T + bq) * SCALE
    k = query @ Wk.T + bk
    v = query @ Wv.T + bv

    def shp(x):
        return x.reshape(T, B * H, HD).transpose(1, 0, 2)

    q, k, v = shp(q), shp(k), shp(v)
    w = np.einsum('bth,bsh->bts', q, k).reshape(B, H, T, T) + attn_mask
    w = np.where(key_padding_mask[:, None, None, :], -np.inf, w)
    w = w - w.max(axis=-1, keepdims=True)
    ew = np.exp(w)
    p = (ew / ew.sum(axis=-1, keepdims=True)).reshape(B * H, T, T)
    o = np.einsum('bts,bsh->bth', p, v.reshape(B * H, T, HD))
    o = o.transpose(1, 0, 2).reshape(T, B, E)
    return (o @ Wo.T + bo).astype(np.float32)


def _prep_inputs(query, attn_mask, Wq, bq, Wk, Wv, Wo, cls):
    """Build the 8 per-core input maps."""
    import ml_dtypes
    bf = ml_dtypes.bfloat16
    add_blocks = [(mb, sb) for mb in range(T // 128) for sb in range(T // 128)
                  if cls[mb, sb] == ADD]
    n_add = len(add_blocks)
    if n_add:
        mskp = np.empty((128, n_add * 128), np.float32)
        for i, (mb, sb) in enumerate(add_blocks):
            blk = attn_mask[mb * 128:(mb + 1) * 128, sb * 128:(sb + 1) * 128]
            mskp[:, i * 128:(i + 1) * 128] = np.ascontiguousarray(blk.T)
    else:
        mskp = np.zeros((128, 128), np.float32)
    bin_blocks = [(mb, sb) for mb in range(T // 128) for sb in range(T // 128)
                  if cls[mb, sb] == ADDBIN]
    if bin_blocks:
        trip = np.empty((128, len(bin_blocks) * 128), bf)
        for i, (mb, sb) in enumerate(bin_blocks):
            blk = attn_mask[mb * 128:(mb + 1) * 128, sb * 128:(sb + 1) * 128]
            trip[:, i * 128:(i + 1) * 128] = (blk.T == 0.0).astype(bf)
    else:
        trip = np.zeros((128, 128), bf)
    ones1 = np.ones((1, 64), bf)
    identity = np.eye(128, dtype=np.float32).astype(bf)

    in_maps = []
    for core in range(NCORES):
        b = core // (NCORES // B)
        jsl = slice((core % (NCORES // B)) * J, (core % (NCORES // B)) * J + J)
        EC_, J_ = E // 128, J

        def sb_layout(wT):  # [E, J] -> SBUF [128, EC*J]
            return np.ascontiguousarray(
                wT.reshape(EC_, 128, J_).transpose(1, 0, 2).reshape(128, EC_ * J_))

        xT_c = np.ascontiguousarray(query[:, b, :].T).astype(bf)
        wq_l = sb_layout((Wq[jsl, :] * np.float32(SCALE)).T)
        wk_l = sb_layout(Wk[jsl, :].T)
        wv_l = sb_layout(Wv[jsl, :].T)
        wqpack = np.ascontiguousarray(wq_l).astype(bf)
        wkvpack = np.concatenate([wk_l, wv_l], axis=1).astype(bf)
        woT = Wo[:, jsl].T  # [J, E]
        wopack = np.ascontiguousarray(
            woT.reshape(J_ // 128, 128, E).transpose(1, 0, 2)
            .reshape(128, (J_ // 128) * E)).astype(bf)
        bq_c = np.ascontiguousarray(
            (bq[jsl] * np.float32(SCALE)).reshape(2, 128).T)
        in_maps.append({
            "xT": xT_c, "wqpack": wqpack, "wkvpack": wkvpack,
            "wopack": wopack, "bqp": bq_c, "ones1": ones1, "msk": mskp,
            "tri": trip, "ident": identity,
        })
    return in_maps


def _kernel_impl(inputs, trace=False, **run_kwargs):
    query = np.asarray(inputs["query"], np.float32)
    attn_mask = np.asarray(inputs["attn_mask"], np.float32)
    kpm = np.asarray(inputs["key_padding_mask"])
    Wq = np.asarray(inputs["Wq"], np.float32)
    bq = np.asarray(inputs["bq"], np.float32)
    Wk = np.asarray(inputs["Wk"], np.float32)
    bk = np.asarray(inputs["bk"], np.float32)
    Wv = np.asarray(inputs["Wv"], np.float32)
    bv = np.asarray(inputs["bv"], np.float32)
    Wo = np.asarray(inputs["Wo"], np.float32)
    bo = np.asarray(inputs["bo"], np.float32)

    # Fast path requires: no key padding, no fully-masked rows, block-
    # classifiable mask with a modest number of additive blocks, and no
    # bk dependence issue (bk shifts are softmax-invariant, always ok).
    cls = _classify_mask(attn_mask)
    fallback = (
        kpm.any()
        or (attn_mask.max(axis=1) <= NEG_THRESH).any()
        or (cls == ADD).sum() > 24 or (cls == ADDBIN).sum() > 24
        or np.isnan(attn_mask).any()
    )
    if fallback:
        return _numpy_ref(query, attn_mask, kpm, Wq, bq, Wk, bk, Wv, bv,
                          Wo, bo), None

    nc = _get_program(T, cls)
    in_maps = _prep_inputs(query, attn_mask, Wq, bq, Wk, Wv, Wo, cls)
    res = run_bass_kernel_spmd(nc, in_maps, core_ids=list(range(NCORES)),
                               trace=trace, **run_kwargs)

    # unshard: sum the 4 row-split partials per batch element (the Wo
    # all-reduce), then add bo and the bv contribution (sum_s p = 1).
    bo_total = bo + Wo @ bv
    out = np.empty((T, B, E), np.float32)
    gsz = NCORES // B
    for b in range(B):
        acc = res.results[b * gsz]["out"].astype(np.float32)
        for c in range(b * gsz + 1, (b + 1) * gsz):
            acc = acc + res.results[c]["out"]
        out[:, b, :] = acc + bo_total[None, :]
    return out, res


def kernel(**inputs):
    out, _ = _kernel_impl(inputs, trace=False)
    return out
